# revision 1
# baseline (speedup 1.0000x reference)
"""Multi-head attention (B=4, T=2048, C=1024, H=16, D=64) on 8 TRN2 cores.

Sharding: core i handles batch b=i//2 and the 8 heads of half hh=i%2.
Each core computes its heads' contribution through the row-sharded output
projection -> partial y [T, C]; host sums the two partials per batch.

Per-core layouts (host pre-arranged):
  xT  [C, T]    = x[b].T
  wq/wk/wv [C, 512]  columns = (local head)*64 + d
  wpt [512, C]  rows  = (local head)*64 + d   (= Wp.T row-slice)
  bp  [C]       bias on even cores, zeros on odd (summed partials)

On-chip dataflow per core:
  qT/kT [2h*64=128, T] via lhsT=w-chunk, rhs=xT-chunk (f32r, N=512)
  v     [s,d] natural via lhsT=xT s-slice, rhs=wv-chunk (N=256)
  scoresT[s,t]: lhsT=kT s-block [64,128], rhs=qT t-tile [64,512],
                2 heads row-tiled (K=64 each, partitions 0-63 / 64-127)
  exp on ScalarE PSUM->SBUF with scale=1/sqrt(C); causal: restrict to the
  valid t-range, one constant [128,128] mask multiply on straddling blocks
  PV: lhsT=[v ; ones] [128,65], rhs=pT -> outT [65,512] PSUM accumulated
  over s-blocks; row 64 = softmax normalizer Z
  normalize: DVE reciprocal(Z) -> gpsimd partition_broadcast -> DVE mult
  y: lhsT=outcatT [c,t-block], rhs=wpt [c, c'] + bias, DMA out
"""

import os
import sys

import numpy as np

for _p in ("/opt/trn_rl_repo", "/root/.axon_site/_ro/trn_rl_repo"):
    if os.path.isdir(_p) and _p not in sys.path:
        sys.path.append(_p)

import concourse.bass as bass
import concourse.bacc as bacc
import concourse.mybir as mybir
import concourse.tile as tile
from concourse.bass_utils import run_bass_kernel_spmd

B, T, C, H, D = 4, 2048, 1024, 16, 64
HL = H // 2          # heads per core
P = 128
NCH = C // P         # 8 c-chunks
NTT = T // 512       # 4 t-tiles of 512
NSB = T // P         # 16 s-blocks of 128
SCALE = 1.0 / 32.0   # 1/sqrt(C)

F32 = mybir.dt.float32
F32R = mybir.dt.float32r


def _build(causal: bool, debug: bool = False) -> bass.Bass:
    nc = bacc.Bacc("TRN2", target_bir_lowering=False, debug=False, num_devices=8)

    xT = nc.dram_tensor("xT", [C, T], F32R, kind="ExternalInput").ap()
    wq_d = nc.dram_tensor("wq", [C, HL * D], F32R, kind="ExternalInput").ap()
    wk_d = nc.dram_tensor("wk", [C, HL * D], F32R, kind="ExternalInput").ap()
    wv_d = nc.dram_tensor("wv", [C, HL * D], F32R, kind="ExternalInput").ap()
    wpt_d = nc.dram_tensor("wpt", [HL * D, C], F32R, kind="ExternalInput").ap()
    bp_d = nc.dram_tensor("bp", [C], F32, kind="ExternalInput").ap()
    y_d = nc.dram_tensor("y", [T, C], F32, kind="ExternalOutput").ap()
    dbg = {}
    if debug:
        dbg["q"] = nc.dram_tensor("dbg_q", [2, P, T], F32, kind="ExternalOutput").ap()
        dbg["k"] = nc.dram_tensor("dbg_k", [2, P, T], F32, kind="ExternalOutput").ap()
        dbg["v"] = nc.dram_tensor("dbg_v", [P, NSB * 4 * (D + 1)], F32, kind="ExternalOutput").ap()
        dbg["oc"] = nc.dram_tensor("dbg_oc", [4, P, T], F32, kind="ExternalOutput").ap()

    with tile.TileContext(nc) as tc:
        _emit(nc, tc, causal, xT, wq_d, wk_d, wv_d, wpt_d, bp_d, y_d, dbg)
    nc.compile()
    return nc


def _emit(nc, tc, causal, xT, wq_d, wk_d, wv_d, wpt_d, bp_d, y_d, dbg={}):
    from contextlib import ExitStack

    ctx = ExitStack()
    with ctx:
        consts = ctx.enter_context(tc.tile_pool(name="consts", bufs=1))
        q_pool = ctx.enter_context(tc.tile_pool(name="qT", bufs=3))
        k_pool = ctx.enter_context(tc.tile_pool(name="kT", bufs=3))
        v_pool = ctx.enter_context(tc.tile_pool(name="v", bufs=2))
        oc_pool = ctx.enter_context(tc.tile_pool(name="outcat", bufs=4))
        p_pool = ctx.enter_context(tc.tile_pool(name="pT", bufs=3))
        z_pool = ctx.enter_context(tc.tile_pool(name="zb", bufs=2))
        rzb_pool = ctx.enter_context(tc.tile_pool(name="rzb", bufs=2))
        psA = ctx.enter_context(tc.tile_pool(name="psA", bufs=2, space="PSUM"))
        psB = ctx.enter_context(tc.tile_pool(name="psB", bufs=2, space="PSUM"))
        pso = ctx.enter_context(tc.tile_pool(name="pso", bufs=2, space="PSUM"))

        # constant [128, 2, 128] additive causal mask: 0 where free>=partition
        # else -1e9 (two copies along the middle dim, one per row-tiled head)
        mask = None
        if causal:
            mask = consts.tile([P, 2, P], F32)
            nc.vector.memset(mask, 0.0)
            for _u in range(2):
                nc.gpsimd.affine_select(
                    out=mask[:, _u, :], in_=mask[:, _u, :],
                    compare_op=mybir.AluOpType.is_ge,
                    fill=-1e9, base=0,
                    pattern=[[1, P]], channel_multiplier=-1,
                )

        ones_bc = consts.tile([P, P], F32R)
        nc.vector.memset(ones_bc.bitcast(F32), 1.0)

        outcat = [oc_pool.tile([P, T], F32R, tag="outcat", name=f"outcat{i}")
                  for i in range(4)]

        inner = ExitStack()
        with inner:
            wq_pool = inner.enter_context(tc.tile_pool(name="wq", bufs=1))
            wk_pool = inner.enter_context(tc.tile_pool(name="wk", bufs=1))
            wv_pool = inner.enter_context(tc.tile_pool(name="wv", bufs=1))
            x_pool = inner.enter_context(tc.tile_pool(name="xh", bufs=1))

            for hg in range(2):  # head-group of 4 heads (= 2 pairs)
                hsl = slice(hg * 4 * D, (hg + 1) * 4 * D)
                wq_t = wq_pool.tile([P, NCH, 4 * D], F32R, tag="wq")
                wk_t = wk_pool.tile([P, NCH, 4 * D], F32R, tag="wk")
                wv_t = wv_pool.tile([P, NCH, 4 * D], F32R, tag="wv")
                for w_t, w_d in ((wq_t, wq_d), (wk_t, wk_d), (wv_t, wv_d)):
                    nc.sync.dma_start(
                        out=w_t,
                        in_=w_d[:, hsl].rearrange("(n p) d -> p n d", p=P),
                    )

                qT2 = [q_pool.tile([P, T], F32R, tag="qT", name=f"qT{i}")
                       for i in range(2)]
                kT2 = [k_pool.tile([P, T], F32R, tag="kT", name=f"kT{i}")
                       for i in range(2)]
                # v: [s-part, s-block, head-in-group, d + ones]
                v_t = v_pool.tile([P, NSB, 4, D + 1], F32R, tag="v")
                nc.vector.memset(v_t[:, :, :, D:].bitcast(F32), 1.0)

                for th in range(2):  # t/s halves of 1024
                    xh = x_pool.tile([P, NCH, 1024], F32R, tag="xh")
                    for c in range(NCH):
                        nc.sync.dma_start(
                            out=xh[:, c, :],
                            in_=xT[c * P:(c + 1) * P, th * 1024:(th + 1) * 1024],
                        )
                    tg = slice(th * 1024, (th + 1) * 1024)
                    # ---- q/k projections ----
                    for pr in range(2):
                        wsl = slice(pr * P, (pr + 1) * P)
                        qps = psA.tile([P, 2, 512], F32, tag="psA", name="qps")
                        kps = psA.tile([P, 2, 512], F32, tag="psA", name="kps")
                        for c in range(NCH):
                            for tt in range(2):
                                nc.tensor.matmul(
                                    qps[:, tt, :], wq_t[:, c, wsl],
                                    xh[:, c, tt * 512:(tt + 1) * 512],
                                    start=c == 0, stop=c == NCH - 1)
                                nc.tensor.matmul(
                                    kps[:, tt, :], wk_t[:, c, wsl],
                                    xh[:, c, tt * 512:(tt + 1) * 512],
                                    start=c == 0, stop=c == NCH - 1)
                        nc.vector.tensor_copy(
                            out=qT2[pr][:, tg],
                            in_=qps.rearrange("p u t -> p (u t)"))
                        nc.vector.tensor_copy(
                            out=kT2[pr][:, tg],
                            in_=kps.rearrange("p u t -> p (u t)"))
                    # ---- v projection (natural [s, d]) ----
                    for sbp in range(4):
                        vps = psB.tile([P, 2, 256], F32, tag="psB", name="vps")
                        for c in range(NCH):
                            for u in range(2):
                                nc.tensor.matmul(
                                    vps[:, u, :],
                                    xh[:, c, (sbp * 2 + u) * P:(sbp * 2 + u + 1) * P],
                                    wv_t[:, c, :],
                                    start=(c == 0 and u == 0), stop=c == NCH - 1)
                        sb0 = th * 8 + sbp * 2
                        nc.vector.tensor_copy(
                            out=v_t[:, sb0:sb0 + 2, :, 0:D],
                            in_=vps.rearrange("p u (h d) -> p u h d", h=4),
                        )

                if dbg and hg == 0:
                    for pr2 in range(2):
                        nc.sync.dma_start(out=dbg["q"][pr2], in_=qT2[pr2].bitcast(F32))
                        nc.sync.dma_start(out=dbg["k"][pr2], in_=kT2[pr2].bitcast(F32))
                    nc.sync.dma_start(
                        out=dbg["v"],
                        in_=v_t.rearrange("p a b c -> p (a b c)").bitcast(F32))

                # ---- attention for this head-group ----
                for pr in range(2):
                    pair = hg * 2 + pr
                    zb = z_pool.tile([P, 3, 512], F32, tag="zb", name="zb")
                    nc.vector.memset(zb, 1.0)
                    for j in range(NTT):
                        nsb_j = 4 * (j + 1) if causal else NSB
                        outp = [pso.tile([D + 1, 512], F32, tag="pso",
                                         name=f"outp{i}") for i in range(2)]
                        def emit_pv(i, lo, last):
                            for u in range(2):
                                nc.tensor.matmul(
                                    outp[u][:, lo:512],
                                    v_t[:, i, pr * 2 + u, :],
                                    pend[i][:, u, lo:512],
                                    start=(i == 0), stop=last,
                                    skip_group_check=True)
                            del pend[i]

                        pend = {}
                        prev = None
                        for i in range(nsb_j):
                            r = i - 4 * j if causal else -1
                            lo = max(r, 0) * P
                            last = i == nsb_j - 1
                            scs = psA.tile([P, 2, 512], F32, tag="psA", name="scs")
                            pts = p_pool.tile([P, 2, 512], F32R, tag="pT", name="pts")
                            pend[i] = pts
                            for u in range(2):
                                dsl = slice(u * D, (u + 1) * D)
                                nc.tensor.matmul(
                                    scs[:, u, :],
                                    kT2[pr][dsl, i * P:(i + 1) * P],
                                    qT2[pr][dsl, j * 512:(j + 1) * 512],
                                    start=True, stop=True)
                            if causal and r >= 0:
                                nc.vector.tensor_add(
                                    scs[:, :, lo:lo + P],
                                    scs[:, :, lo:lo + P],
                                    mask)
                            nc.scalar.activation(
                                out=pts[:, :, lo:512],
                                in_=scs[:, :, lo:512],
                                func=mybir.ActivationFunctionType.Exp,
                                scale=SCALE)
                            if prev is not None:
                                emit_pv(*prev)
                            prev = (i, lo, last)
                        if prev is not None:
                            emit_pv(*prev)
                        for u in range(2):
                            # raw (unnormalized) head output + Z row gather
                            nc.vector.tensor_copy(
                                out=outcat[pair][u * D:(u + 1) * D,
                                                 j * 512:(j + 1) * 512],
                                in_=outp[u][0:D, :])
                            idx = j * 2 + u
                            nc.vector.tensor_copy(
                                out=zb[32 * (idx // 3):32 * (idx // 3) + 1,
                                       idx % 3, :],
                                in_=outp[u][D:D + 1, :])
                    # batched normalizer: one reciprocal for all 8 (j, u)
                    # rows, then per-row broadcast via K=1 matmul into PSUM
                    rzb_all = z_pool.tile([P, 3, 512], F32R, tag="zb", name="rz_all")
                    with nc.allow_low_precision(reason="softmax normalizer"):
                        nc.vector.reciprocal(out=rzb_all, in_=zb)
                    for j in range(NTT):
                        for u in range(2):
                            idx = j * 2 + u
                            k0 = 32 * (idx // 3)
                            bps = pso.tile([P, 512], F32, tag="pso", name="bps")
                            nc.tensor.matmul(
                                bps,
                                ones_bc[k0:k0 + 1, :],
                                rzb_all[k0:k0 + 1, idx % 3, :],
                                start=True, stop=True)
                            osl = outcat[pair][u * D:(u + 1) * D,
                                               j * 512:(j + 1) * 512]
                            nc.vector.tensor_mul(
                                osl, osl.bitcast(F32),
                                bps[u * D:(u + 1) * D, :])

        if dbg:
            for q2 in range(4):
                nc.sync.dma_start(out=dbg["oc"][q2], in_=outcat[q2].bitcast(F32))

        # ---- output projection ----
        wpt_pool = ctx.enter_context(tc.tile_pool(name="wpt", bufs=4))
        bpb_pool = ctx.enter_context(tc.tile_pool(name="bpb", bufs=1))
        yst_pool = ctx.enter_context(tc.tile_pool(name="yst", bufs=3))
        wpt_t = [wpt_pool.tile([P, C], F32R, tag="wpt", name=f"wpt{i}") for i in range(4)]
        for q in range(4):
            nc.sync.dma_start(out=wpt_t[q], in_=wpt_d[q * P:(q + 1) * P, :])
        bpb = bpb_pool.tile([P, C], F32)
        nc.sync.dma_start(
            out=bpb,
            in_=bass.AP(tensor=bp_d.tensor, offset=0, ap=[[0, P], [1, C]]),
        )
        for m in range(T // P):
            for n in range(2):
                yps = psB.tile([P, 512], F32, tag="psB", name="yps")
                for q in range(4):
                    nc.tensor.matmul(
                        yps,
                        outcat[q][:, m * P:(m + 1) * P],
                        wpt_t[q][:, n * 512:(n + 1) * 512],
                        start=(q == 0), stop=(q == 3))
                yt = yst_pool.tile([P, 512], F32, tag="yst", name="yt")
                nc.vector.tensor_add(yt, yps, bpb[:, n * 512:(n + 1) * 512])
                nc.sync.dma_start(
                    out=y_d[m * P:(m + 1) * P, n * 512:(n + 1) * 512],
                    in_=yt)


_NC_CACHE = {}
LAST_RESULTS = None


def kernel(x, Wq, Wk, Wv, Wp, bp, is_masked, **_unused):
    global LAST_RESULTS
    x = np.asarray(x, np.float32)
    Wq = np.asarray(Wq, np.float32)
    Wk = np.asarray(Wk, np.float32)
    Wv = np.asarray(Wv, np.float32)
    Wp = np.asarray(Wp, np.float32)
    bp = np.asarray(bp, np.float32)
    causal = bool(np.asarray(is_masked).item())

    if causal not in _NC_CACHE:
        _NC_CACHE[causal] = _build(causal)
    nc = _NC_CACHE[causal]

    # host-side layout prep
    wq_r = np.ascontiguousarray(Wq.transpose(1, 0, 2).reshape(C, H * D))
    wk_r = np.ascontiguousarray(Wk.transpose(1, 0, 2).reshape(C, H * D))
    wv_r = np.ascontiguousarray(Wv.transpose(1, 0, 2).reshape(C, H * D))
    wpt = np.ascontiguousarray(Wp.T)
    zeros = np.zeros_like(bp)

    xTs = [np.ascontiguousarray(x[b].T) for b in range(B)]
    in_maps = []
    for core in range(8):
        b, hh = core // 2, core % 2
        csl = slice(hh * HL * D, (hh + 1) * HL * D)
        in_maps.append({
            "xT": xTs[b],
            "wq": np.ascontiguousarray(wq_r[:, csl]),
            "wk": np.ascontiguousarray(wk_r[:, csl]),
            "wv": np.ascontiguousarray(wv_r[:, csl]),
            "wpt": np.ascontiguousarray(wpt[csl, :]),
            "bp": bp if hh == 0 else zeros,
        })

    trace = bool(int(os.environ.get("KERNEL_TRACE", "0")))
    res = run_bass_kernel_spmd(
        nc, in_maps, core_ids=list(range(8)), trace=trace)
    LAST_RESULTS = res

    y = np.empty((B, T, C), np.float32)
    for b in range(B):
        y[b] = res.results[2 * b]["y"] + res.results[2 * b + 1]["y"]
    return y



# revision 18
# speedup vs baseline: 1.3917x; 1.3917x over previous
"""Multi-head attention (B=4, T=2048, C=1024, H=16, D=64) on 8 TRN2 cores.

Sharding: core i handles batch b=i//2 and the 8 heads of half hh=i%2.
Each core computes its heads' contribution through the row-sharded output
projection -> partial y [T, C]; host sums the two partials per batch.

v2: bf16 compute (FWL weight loads, no fp32 power throttle), software
pipelining to keep the PE HAM-warm:
  P1: dense q/k/v projections for head-group 0 (pairs 0,1)
  P2: attention pairs 0,1 with head-group 1's projection matmuls
      interleaved as PE filler between score steps (exp on ScalarE is
      the attention bottleneck; filler keeps the PE from micro-idling)
  P3: attention pairs 2,3 (normalize of earlier pairs interleaved)
  tail: last normalize + output projection
Causal mask applied post-exp on GpSimd (affine_select fill=0), score
matmuls N-trimmed on diagonal blocks, softmax normalizer via
reciprocal_approx_fast + K=2 selector broadcast matmul.

Per-core layouts (host pre-arranged, bf16):
  xT  [C, T]    = x[b].T
  wq/wk/wv [C, 512]  columns = (local head)*64 + d
  wpt [512, C]  rows  = (local head)*64 + d   (= Wp.T row-slice)
  bp  [C] f32   bias on even cores, zeros on odd (summed partials)
"""

import os
import sys

import numpy as np

for _p in ("/opt/trn_rl_repo", "/root/.axon_site/_ro/trn_rl_repo"):
    if os.path.isdir(_p) and _p not in sys.path:
        sys.path.append(_p)

import ml_dtypes

import concourse.bass as bass
import concourse.bacc as bacc
import concourse.mybir as mybir
import concourse.tile as tile
from concourse.bass_utils import run_bass_kernel_spmd

B, T, C, H, D = 4, 2048, 1024, 16, 64
HL = H // 2          # heads per core
P = 128
NCH = C // P         # 8 c-chunks
NTT = T // 512       # 4 t-tiles of 512
NSB = T // P         # 16 s-blocks of 128
SCALE = 1.0 / 32.0   # 1/sqrt(C)

F32 = mybir.dt.float32
F32R = mybir.dt.float32r
BF16 = mybir.dt.bfloat16
NPBF16 = ml_dtypes.bfloat16


def _build(causal: bool) -> bass.Bass:
    nc = bacc.Bacc("TRN2", target_bir_lowering=False, debug=False, num_devices=8)

    xT = nc.dram_tensor("xT", [C, T], BF16, kind="ExternalInput").ap()
    wq_d = nc.dram_tensor("wq", [C, HL * D], BF16, kind="ExternalInput").ap()
    wk_d = nc.dram_tensor("wk", [C, HL * D], BF16, kind="ExternalInput").ap()
    wv_d = nc.dram_tensor("wv", [C, HL * D], BF16, kind="ExternalInput").ap()
    wpt_d = nc.dram_tensor("wpt", [HL * D, C], BF16, kind="ExternalInput").ap()
    bp_d = nc.dram_tensor("bp", [C], F32, kind="ExternalInput").ap()
    sel_d = nc.dram_tensor("sel", [64, P], BF16, kind="ExternalInput").ap()
    y_d = nc.dram_tensor("y", [T, C], F32, kind="ExternalOutput").ap()

    with tile.TileContext(nc) as tc:
        _emit(nc, tc, causal, xT, wq_d, wk_d, wv_d, wpt_d, bp_d, sel_d, y_d)
    nc.compile()
    return nc


def _emit(nc, tc, causal, xT, wq_d, wk_d, wv_d, wpt_d, bp_d, sel_d, y_d):
    from contextlib import ExitStack

    ctx = ExitStack()
    with ctx:
        consts = ctx.enter_context(tc.tile_pool(name="consts", bufs=1))
        x_pool = ctx.enter_context(tc.tile_pool(name="xh", bufs=1))
        wq_pool = ctx.enter_context(tc.tile_pool(name="wq", bufs=2))
        wk_pool = ctx.enter_context(tc.tile_pool(name="wk", bufs=2))
        wv_pool = ctx.enter_context(tc.tile_pool(name="wv", bufs=2))
        q_pool = ctx.enter_context(tc.tile_pool(name="qT", bufs=4))
        k_pool = ctx.enter_context(tc.tile_pool(name="kT", bufs=4))
        v_pool = ctx.enter_context(tc.tile_pool(name="v", bufs=2))
        oc_pool = ctx.enter_context(tc.tile_pool(name="outcat", bufs=4))
        p_pool = ctx.enter_context(tc.tile_pool(name="pT", bufs=4))
        z_pool = ctx.enter_context(tc.tile_pool(name="zb", bufs=2))
        rz_pool = ctx.enter_context(tc.tile_pool(name="rz", bufs=2))
        rzs_pool = ctx.enter_context(tc.tile_pool(name="rzs", bufs=1))
        yst_pool = ctx.enter_context(tc.tile_pool(name="yst", bufs=3))
        wpt_pool = ctx.enter_context(tc.tile_pool(name="wpt", bufs=4))
        bpb_pool = ctx.enter_context(tc.tile_pool(name="bpb", bufs=1))
        psA = ctx.enter_context(tc.tile_pool(name="psA", bufs=2, space="PSUM"))
        pso = ctx.enter_context(tc.tile_pool(name="pso", bufs=2, space="PSUM"))
        psP = ctx.enter_context(tc.tile_pool(name="psP", bufs=2, space="PSUM"))

        # Normalizer broadcast selector (host constant): row 0 -> out rows
        # 0-63 (u=0), row 32 -> out rows 64-127 (u=1); all other rows zero
        # so garbage in the unused rz partitions is multiplied by 0.
        sel = consts.tile([64, P], BF16)
        nc.sync.dma_start(out=sel, in_=sel_d)

        # whole xT resident in SBUF (used by both head-groups); DMA in
        # t-window order so the first projection group starts early
        xh = x_pool.tile([P, NCH, T], BF16, tag="xh")
        for w in range(4):
            for c in range(NCH):
                nc.sync.dma_start(
                    out=xh[:, c, w * 512:(w + 1) * 512],
                    in_=xT[c * P:(c + 1) * P, w * 512:(w + 1) * 512])

        qT2 = [None] * 4   # per pair [128 (2 heads x 64d), T] bf16
        kT2 = [None] * 4
        v_t = [None] * 2   # per head-group [128 s, NSB, 4, D+1] bf16
        outcat = [oc_pool.tile([P, T], BF16, tag="outcat", name=f"outcat{i}")
                  for i in range(4)]
        zb = [None] * 4    # per pair [128, NTT, 512] f32; rows 0(u0)/1(u1)
        rz = [None] * 4

        # ---------------- projection emission (generator) ----------------
        def emit_weight_dmas(hg):
            hsl = slice(hg * 4 * D, (hg + 1) * 4 * D)
            wq_t = wq_pool.tile([P, NCH, 4 * D], BF16, tag="wq")
            wk_t = wk_pool.tile([P, NCH, 4 * D], BF16, tag="wk")
            wv_t = wv_pool.tile([P, NCH, 4 * D], BF16, tag="wv")
            for w_t, w_d in ((wq_t, wq_d), (wk_t, wk_d), (wv_t, wv_d)):
                nc.sync.dma_start(
                    out=w_t,
                    in_=w_d[:, hsl].rearrange("(n p) d -> p n d", p=P),
                )
            return wq_t, wk_t, wv_t

        def proj_groups(hg, wq_t, wk_t, wv_t, copy_engine):
            """Yield once per emitted matmul group (PE filler granularity)."""
            for pr in range(2):
                pair = hg * 2 + pr
                qT2[pair] = q_pool.tile([P, T], BF16, tag="qT",
                                        name=f"qT{pair}")
                kT2[pair] = k_pool.tile([P, T], BF16, tag="kT",
                                        name=f"kT{pair}")
            v_t[hg] = v_pool.tile([P, NSB, 4, D + 1], BF16, tag="v",
                                  name=f"v{hg}")
            nc.vector.memset(v_t[hg][:, :, :, D:], 1.0)
            wsl = [slice(pr * P, (pr + 1) * P) for pr in range(2)]
            for th in range(2):
                for pr in range(2):
                    for w_t, dst in ((wq_t, qT2[hg * 2 + pr]),
                                     (wk_t, kT2[hg * 2 + pr])):
                        for tt in range(2):
                            t0 = th * 1024 + tt * 512
                            ps = psP.tile([P, 512], F32, tag="psP", name="qk")
                            for c in range(NCH):
                                nc.tensor.matmul(
                                    ps, w_t[:, c, wsl[pr]],
                                    xh[:, c, t0:t0 + 512],
                                    start=c == 0, stop=c == NCH - 1)
                            copy_engine(out=dst[:, t0:t0 + 512], in_=ps)
                            yield
                # v projection (natural [s, d]) for this th
                for sbp in range(4):
                    vps = psP.tile([P, 2, 256], F32, tag="psP", name="vps")
                    for c in range(NCH):
                        for u in range(2):
                            s0 = th * 1024 + (sbp * 2 + u) * P
                            nc.tensor.matmul(
                                vps[:, u, :],
                                xh[:, c, s0:s0 + P],
                                wv_t[:, c, :],
                                start=(c == 0 and u == 0), stop=c == NCH - 1)
                    sb0 = th * 8 + sbp * 2
                    nc.vector.tensor_copy(
                        out=v_t[hg][:, sb0:sb0 + 2, :, 0:D],
                        in_=vps.rearrange("p u (h d) -> p u h d", h=4))
                    yield

        def act_copy(out, in_):
            nc.scalar.copy(out=out, in_=in_)

        def dve_copy(out, in_):
            nc.vector.tensor_copy(out=out, in_=in_)

        # ---------------- normalize emission ----------------
        def emit_normalize(pair):
            rz[pair] = rz_pool.tile([P, NTT, 512], BF16, tag="rz",
                                    name=f"rz{pair}")
            scratch = rzs_pool.tile([64, NTT, 512], F32, tag="rzs")
            nc.vector.reciprocal_approx_fast(
                out=scratch, in_=zb[pair][0:64, :, :])
            nc.vector.tensor_copy(out=rz[pair][0:64, :, :], in_=scratch)
            for j in range(NTT):
                bps = psP.tile([P, 512], F32, tag="psP", name="bps")
                nc.tensor.matmul(bps, sel, rz[pair][0:64, j, :],
                                 start=True, stop=True)
                osl = outcat[pair][:, j * 512:(j + 1) * 512]
                nc.vector.tensor_mul(osl, osl, bps)

        # ---------------- attention ----------------
        def attention_pair(pair, filler, fill_every):
            hg, pr = pair // 2, pair % 2
            zb[pair] = z_pool.tile([P, NTT, 512], F32, tag="zb",
                                   name=f"zb{pair}")
            # rows 1-31/33-63 are never written but feed the (zero-weighted)
            # reciprocal input; keep them finite
            nc.vector.memset(zb[pair][0:64, :, :], 1.0)
            step = 0
            for j in range(NTT):
                nsb_j = 4 * (j + 1) if causal else NSB
                outp = [pso.tile([D + 1, 512], F32, tag="pso",
                                 name=f"outp{i}") for i in range(2)]

                def emit_pv(i, lo, last):
                    for u in range(2):
                        nc.tensor.matmul(
                            outp[u][:, lo:512],
                            v_t[hg][:, i, pr * 2 + u, :],
                            pend[i][:, u, lo:512],
                            start=(i == 0), stop=last,
                            skip_group_check=True)
                    del pend[i]

                pend = {}
                prev = None
                for i in range(nsb_j):
                    r = i - 4 * j if causal else -1
                    lo = max(r, 0) * P
                    last = i == nsb_j - 1
                    scs = psA.tile([P, 2, 512], F32, tag="psA", name="scs")
                    pts = p_pool.tile([P, 2, 512], BF16, tag="pT", name="pts")
                    pend[i] = pts
                    for u in range(2):
                        dsl = slice(u * D, (u + 1) * D)
                        nc.tensor.matmul(
                            scs[:, u, lo:512],
                            kT2[pair][dsl, i * P:(i + 1) * P],
                            qT2[pair][dsl, j * 512 + lo:(j + 1) * 512],
                            start=True, stop=True)
                    nc.scalar.activation(
                        out=pts[:, :, lo:512],
                        in_=scs[:, :, lo:512],
                        func=mybir.ActivationFunctionType.Exp,
                        scale=SCALE)
                    if causal and r >= 0:
                        # zero the upper triangle of the diagonal block
                        # post-exp (GpSimd; keeps DVE/ScalarE free)
                        for u in range(2):
                            nc.gpsimd.affine_select(
                                out=pts[:, u, lo:lo + P],
                                in_=pts[:, u, lo:lo + P],
                                compare_op=mybir.AluOpType.is_ge,
                                fill=0.0, base=0,
                                pattern=[[1, P]], channel_multiplier=-1,
                            )
                    if prev is not None:
                        emit_pv(*prev)
                    prev = (i, lo, last)
                    step += 1
                    if filler is not None and fill_every and \
                            step % fill_every == 0:
                        next(filler, None)
                if prev is not None:
                    emit_pv(*prev)
                for u in range(2):
                    nc.vector.tensor_copy(
                        out=outcat[pair][u * D:(u + 1) * D,
                                         j * 512:(j + 1) * 512],
                        in_=outp[u][0:D, :])
                    nc.vector.tensor_copy(
                        out=zb[pair][32 * u:32 * u + 1, j, :],
                        in_=outp[u][D:D + 1, :])

        # ================= schedule =================
        # P1: dense projections for head-group 0 (ScalarE idle -> ACT copies)
        # head-group 1 weight DMAs prefetched here so P2 filler never waits
        w0 = emit_weight_dmas(0)
        w1 = emit_weight_dmas(1)
        for _ in proj_groups(0, *w0, act_copy):
            pass

        # P2: attention pairs 0,1 with head-group 1 projections as filler
        filler = proj_groups(1, *w1, dve_copy)
        attention_pair(0, filler, 3)
        attention_pair(1, filler, 3)
        # drain any remaining filler groups
        for _ in filler:
            pass
        emit_normalize(0)

        # P3: attention pairs 2,3; normalize earlier pairs in the gaps
        attention_pair(2, None, 0)
        emit_normalize(1)
        attention_pair(3, None, 0)
        emit_normalize(2)
        emit_normalize(3)

        # tail: output projection y = outcat_norm.T @ wpt + bp
        wpt_t = [wpt_pool.tile([P, C], BF16, tag="wpt", name=f"wpt{i}")
                 for i in range(4)]
        for q in range(4):
            nc.sync.dma_start(out=wpt_t[q], in_=wpt_d[q * P:(q + 1) * P, :])
        bpb = bpb_pool.tile([P, C], F32)
        nc.sync.dma_start(
            out=bpb,
            in_=bass.AP(tensor=bp_d.tensor, offset=0, ap=[[0, P], [1, C]]),
        )
        for m in range(T // P):
            for n in range(2):
                yps = psP.tile([P, 512], F32, tag="psP", name="yps")
                for q in range(4):
                    nc.tensor.matmul(
                        yps,
                        outcat[q][:, m * P:(m + 1) * P],
                        wpt_t[q][:, n * 512:(n + 1) * 512],
                        start=(q == 0), stop=(q == 3))
                yt = yst_pool.tile([P, 512], F32, tag="yst", name="yt")
                nc.vector.tensor_add(yt, yps, bpb[:, n * 512:(n + 1) * 512])
                nc.sync.dma_start(
                    out=y_d[m * P:(m + 1) * P, n * 512:(n + 1) * 512],
                    in_=yt)


_NC_CACHE = {}
LAST_RESULTS = None


def kernel(x, Wq, Wk, Wv, Wp, bp, is_masked, **_unused):
    global LAST_RESULTS
    x = np.asarray(x, np.float32)
    Wq = np.asarray(Wq, np.float32)
    Wk = np.asarray(Wk, np.float32)
    Wv = np.asarray(Wv, np.float32)
    Wp = np.asarray(Wp, np.float32)
    bp = np.asarray(bp, np.float32)
    causal = bool(np.asarray(is_masked).item())

    if causal not in _NC_CACHE:
        _NC_CACHE[causal] = _build(causal)
    nc = _NC_CACHE[causal]

    # host-side layout prep (bf16)
    wq_r = np.ascontiguousarray(
        Wq.transpose(1, 0, 2).reshape(C, H * D)).astype(NPBF16)
    wk_r = np.ascontiguousarray(
        Wk.transpose(1, 0, 2).reshape(C, H * D)).astype(NPBF16)
    wv_r = np.ascontiguousarray(
        Wv.transpose(1, 0, 2).reshape(C, H * D)).astype(NPBF16)
    wpt = np.ascontiguousarray(Wp.T).astype(NPBF16)
    zeros = np.zeros_like(bp)

    sel = np.zeros((64, P), np.float32)
    sel[0, 0:64] = 1.0
    sel[32, 64:128] = 1.0
    sel = sel.astype(NPBF16)

    xTs = [np.ascontiguousarray(x[b].T).astype(NPBF16) for b in range(B)]
    in_maps = []
    for core in range(8):
        b, hh = core // 2, core % 2
        csl = slice(hh * HL * D, (hh + 1) * HL * D)
        in_maps.append({
            "xT": xTs[b],
            "wq": np.ascontiguousarray(wq_r[:, csl]),
            "wk": np.ascontiguousarray(wk_r[:, csl]),
            "wv": np.ascontiguousarray(wv_r[:, csl]),
            "wpt": np.ascontiguousarray(wpt[csl, :]),
            "bp": bp if hh == 0 else zeros,
            "sel": sel,
        })

    trace = bool(int(os.environ.get("KERNEL_TRACE", "0")))
    res = run_bass_kernel_spmd(
        nc, in_maps, core_ids=list(range(8)), trace=trace)
    LAST_RESULTS = res

    y = np.empty((B, T, C), np.float32)
    for b in range(B):
        y[b] = res.results[2 * b]["y"] + res.results[2 * b + 1]["y"]
    return y


# revision 23
# speedup vs baseline: 1.5628x; 1.1229x over previous
"""Multi-head attention (B=4, T=2048, C=1024, H=16, D=64) on 8 TRN2 cores.

Sharding: core i handles batch b=i//2 and the 8 heads of half hh=i%2.
Each core computes its heads' contribution through the row-sharded output
projection -> partial y [T, C]; host sums the two partials per batch.

v2: bf16 compute (FWL weight loads, no fp32 power throttle), software
pipelining to keep the PE HAM-warm:
  P1: dense q/k/v projections for head-group 0 (pairs 0,1)
  P2: attention pairs 0,1 with head-group 1's projection matmuls
      interleaved as PE filler between score steps (exp on ScalarE is
      the attention bottleneck; filler keeps the PE from micro-idling)
  P3: attention pairs 2,3 (normalize of earlier pairs interleaved)
  tail: last normalize + output projection
Causal mask applied post-exp on GpSimd (affine_select fill=0), score
matmuls N-trimmed on diagonal blocks, softmax normalizer via
reciprocal_approx_fast + K=2 selector broadcast matmul.

Per-core layouts (host pre-arranged, bf16):
  xT  [C, T]    = x[b].T
  wq/wk/wv [C, 512]  columns = (local head)*64 + d
  wpt [512, C]  rows  = (local head)*64 + d   (= Wp.T row-slice)
  bp  [C] f32   bias on even cores, zeros on odd (summed partials)
"""

import os
import sys

import numpy as np

for _p in ("/opt/trn_rl_repo", "/root/.axon_site/_ro/trn_rl_repo"):
    if os.path.isdir(_p) and _p not in sys.path:
        sys.path.append(_p)

import ml_dtypes

import concourse.bass as bass
import concourse.bacc as bacc
import concourse.mybir as mybir
import concourse.tile as tile
from concourse.bass_utils import run_bass_kernel_spmd

B, T, C, H, D = 4, 2048, 1024, 16, 64
HL = H // 2          # heads per core
P = 128
NCH = C // P         # 8 c-chunks
NTT = T // 512       # 4 t-tiles of 512
NSB = T // P         # 16 s-blocks of 128
SCALE = 1.0 / 32.0   # 1/sqrt(C)

F32 = mybir.dt.float32
F32R = mybir.dt.float32r
BF16 = mybir.dt.bfloat16
NPBF16 = ml_dtypes.bfloat16


def _build(causal: bool) -> bass.Bass:
    nc = bacc.Bacc("TRN2", target_bir_lowering=False, debug=False, num_devices=8)

    xT = nc.dram_tensor("xT", [C, T], BF16, kind="ExternalInput").ap()
    wq_d = nc.dram_tensor("wq", [C, HL * D], BF16, kind="ExternalInput").ap()
    wk_d = nc.dram_tensor("wk", [C, HL * D], BF16, kind="ExternalInput").ap()
    wv_d = nc.dram_tensor("wv", [C, HL * D], BF16, kind="ExternalInput").ap()
    wpt_d = nc.dram_tensor("wpt", [HL * D, C], BF16, kind="ExternalInput").ap()
    bp_d = nc.dram_tensor("bp", [C], F32, kind="ExternalInput").ap()
    sel_d = nc.dram_tensor("sel", [64, P], BF16, kind="ExternalInput").ap()
    y_d = nc.dram_tensor("y", [T, C], F32, kind="ExternalOutput").ap()

    with tile.TileContext(nc) as tc:
        _emit(nc, tc, causal, xT, wq_d, wk_d, wv_d, wpt_d, bp_d, sel_d, y_d)
    nc.compile()
    return nc


def _emit(nc, tc, causal, xT, wq_d, wk_d, wv_d, wpt_d, bp_d, sel_d, y_d):
    from collections import deque
    from contextlib import ExitStack

    ctx = ExitStack()
    with ctx:
        consts = ctx.enter_context(tc.tile_pool(name="consts", bufs=1))
        x_pool = ctx.enter_context(tc.tile_pool(name="xh", bufs=1))
        wq_pool = ctx.enter_context(tc.tile_pool(name="wq", bufs=2))
        wk_pool = ctx.enter_context(tc.tile_pool(name="wk", bufs=2))
        wv_pool = ctx.enter_context(tc.tile_pool(name="wv", bufs=2))
        q_pool = ctx.enter_context(tc.tile_pool(name="qT", bufs=4))
        k_pool = ctx.enter_context(tc.tile_pool(name="kT", bufs=4))
        v_pool = ctx.enter_context(tc.tile_pool(name="v", bufs=2))
        oc_pool = ctx.enter_context(tc.tile_pool(name="outcat", bufs=4))
        p_pool = ctx.enter_context(tc.tile_pool(name="pT", bufs=4))
        z_pool = ctx.enter_context(tc.tile_pool(name="zb", bufs=2))
        rz_pool = ctx.enter_context(tc.tile_pool(name="rz", bufs=2))
        rzs_pool = ctx.enter_context(tc.tile_pool(name="rzs", bufs=1))
        rzs3_pool = ctx.enter_context(tc.tile_pool(name="rzs3", bufs=2))
        yst_pool = ctx.enter_context(tc.tile_pool(name="yst", bufs=3))
        wpt_pool = ctx.enter_context(tc.tile_pool(name="wpt", bufs=4))
        bpb_pool = ctx.enter_context(tc.tile_pool(name="bpb", bufs=1))
        psA = ctx.enter_context(tc.tile_pool(name="psA", bufs=2, space="PSUM"))
        pso = ctx.enter_context(tc.tile_pool(name="pso", bufs=2, space="PSUM"))
        psP = ctx.enter_context(tc.tile_pool(name="psP", bufs=2, space="PSUM"))

        # Normalizer broadcast selector (host constant): row 0 -> out rows
        # 0-63 (u=0), row 32 -> out rows 64-127 (u=1); all other rows zero
        # so garbage in the unused rz partitions is multiplied by 0.
        sel = consts.tile([64, P], BF16)
        nc.sync.dma_start(out=sel, in_=sel_d)

        # ---- input DMAs, ordered so the first projection group (needs
        # wq(hg0) + x window 0) is ready ASAP; wpt/bpb early so the output
        # projection interleaved into pair 3 never waits ----
        xh = x_pool.tile([P, NCH, T], BF16, tag="xh")
        wts = {}
        for hg in range(2):
            wq_t = wq_pool.tile([P, NCH, 4 * D], BF16, tag="wq",
                                name=f"wq{hg}")
            wk_t = wk_pool.tile([P, NCH, 4 * D], BF16, tag="wk",
                                name=f"wk{hg}")
            wv_t = wv_pool.tile([P, NCH, 4 * D], BF16, tag="wv",
                                name=f"wv{hg}")
            wts[hg] = (wq_t, wk_t, wv_t)

        def dma_w(hg, idx):
            hsl = slice(hg * 4 * D, (hg + 1) * 4 * D)
            w_d = (wq_d, wk_d, wv_d)[idx]
            nc.sync.dma_start(
                out=wts[hg][idx],
                in_=w_d[:, hsl].rearrange("(n p) d -> p n d", p=P))

        def dma_x(w):
            for c in range(NCH):
                nc.sync.dma_start(
                    out=xh[:, c, w * 512:(w + 1) * 512],
                    in_=xT[c * P:(c + 1) * P, w * 512:(w + 1) * 512])

        dma_w(0, 0)               # wq(hg0)
        dma_x(0)
        dma_w(0, 1)               # wk(hg0)
        dma_x(1)
        dma_w(0, 2)               # wv(hg0)
        dma_x(2)
        dma_x(3)
        for idx in range(3):
            dma_w(1, idx)
        wpt_t = [wpt_pool.tile([P, C], BF16, tag="wpt", name=f"wpt{i}")
                 for i in range(4)]
        for q in range(4):
            nc.scalar.dma_start(out=wpt_t[q], in_=wpt_d[q * P:(q + 1) * P, :])
        bpb = bpb_pool.tile([P, C], F32)
        nc.scalar.dma_start(
            out=bpb,
            in_=bass.AP(tensor=bp_d.tensor, offset=0, ap=[[0, P], [1, C]]),
        )

        qT2 = [None] * 4   # per pair [128 (2 heads x 64d), T] bf16
        kT2 = [None] * 4
        v_t = [None] * 2   # per head-group [128 s, NSB, 4, D+1] bf16
        outcat = [oc_pool.tile([P, T], BF16, tag="outcat", name=f"outcat{i}")
                  for i in range(4)]
        zb = [None] * 4    # per pair [128, NTT, 512] f32; rows 0(u0)/32(u1)
        rz = [None] * 4

        # ---------------- projection pieces (filler-granular) ----------
        def alloc_proj(hg):
            for pr in range(2):
                pair = hg * 2 + pr
                qT2[pair] = q_pool.tile([P, T], BF16, tag="qT",
                                        name=f"qT{pair}")
                kT2[pair] = k_pool.tile([P, T], BF16, tag="kT",
                                        name=f"kT{pair}")
            v_t[hg] = v_pool.tile([P, NSB, 4, D + 1], BF16, tag="v",
                                  name=f"v{hg}")
            nc.vector.memset(v_t[hg][:, :, :, D:], 1.0)

        def qk_group(hg, th, pr, which, tt, on_act):
            w_t = wts[hg][which]
            dst = (qT2 if which == 0 else kT2)[hg * 2 + pr]
            t0 = th * 1024 + tt * 512
            ps = psP.tile([P, 512], F32, tag="psP", name="qk")
            for c in range(NCH):
                nc.tensor.matmul(
                    ps, w_t[:, c, pr * P:(pr + 1) * P],
                    xh[:, c, t0:t0 + 512],
                    start=c == 0, stop=c == NCH - 1)
            if on_act:
                nc.scalar.copy(out=dst[:, t0:t0 + 512], in_=ps)
            else:
                nc.vector.tensor_copy(out=dst[:, t0:t0 + 512], in_=ps)

        def v_group(hg, g):   # g in 0..7 covers s [g*256, (g+1)*256)
            th, sbp = g // 4, g % 4
            wv_t = wts[hg][2]
            vps = psP.tile([P, 2, 256], F32, tag="psP", name="vps")
            for c in range(NCH):
                for u in range(2):
                    s0 = th * 1024 + (sbp * 2 + u) * P
                    nc.tensor.matmul(
                        vps[:, u, :],
                        xh[:, c, s0:s0 + P],
                        wts[hg][2][:, c, :],
                        start=(c == 0 and u == 0), stop=c == NCH - 1)
            sb0 = th * 8 + sbp * 2
            nc.vector.tensor_copy(
                out=v_t[hg][:, sb0:sb0 + 2, :, 0:D],
                in_=vps.rearrange("p u (h d) -> p u h d", h=4))

        # ---------------- normalize ----------------
        def emit_normalize(pair):
            rz[pair] = rz_pool.tile([P, NTT, 512], BF16, tag="rz",
                                    name=f"rz{pair}")
            scratch = rzs_pool.tile([64, NTT, 512], F32, tag="rzs")
            nc.vector.reciprocal_approx_fast(
                out=scratch, in_=zb[pair][0:64, :, :])
            nc.vector.tensor_copy(out=rz[pair][0:64, :, :], in_=scratch)
            for j in range(NTT):
                bps = psP.tile([P, 512], F32, tag="psP", name="bps")
                nc.tensor.matmul(bps, sel, rz[pair][0:64, j, :],
                                 start=True, stop=True)
                osl = outcat[pair][:, j * 512:(j + 1) * 512]
                nc.vector.tensor_mul(osl, osl, bps)

        def norm3_j(j):
            scratch = rzs3_pool.tile([64, 512], F32, tag="rzs3")
            nc.vector.reciprocal_approx_fast(
                out=scratch, in_=zb[3][0:64, j, :])
            nc.vector.tensor_copy(out=rz[3][0:64, j, :], in_=scratch)
            bps = psP.tile([P, 512], F32, tag="psP", name="bps")
            nc.tensor.matmul(bps, sel, rz[3][0:64, j, :],
                             start=True, stop=True)
            osl = outcat[3][:, j * 512:(j + 1) * 512]
            nc.vector.tensor_mul(osl, osl, bps)

        ydma_flip = [0]

        def yproj_group(m, n):
            yps = psP.tile([P, 512], F32, tag="psP", name="yps")
            for q in range(4):
                nc.tensor.matmul(
                    yps,
                    outcat[q][:, m * P:(m + 1) * P],
                    wpt_t[q][:, n * 512:(n + 1) * 512],
                    start=(q == 0), stop=(q == 3))
            yt = yst_pool.tile([P, 512], F32, tag="yst", name="yt")
            nc.vector.tensor_add(yt, yps, bpb[:, n * 512:(n + 1) * 512])
            eng = nc.sync if ydma_flip[0] % 2 == 0 else nc.scalar
            ydma_flip[0] += 1
            eng.dma_start(
                out=y_d[m * P:(m + 1) * P, n * 512:(n + 1) * 512],
                in_=yt)

        # ---------------- attention ----------------
        def attention_pair(pair, fq, fill_every, on_j_done=None):
            hg, pr = pair // 2, pair % 2
            zb[pair] = z_pool.tile([P, NTT, 512], F32, tag="zb",
                                   name=f"zb{pair}")
            # rows 1-31/33-63 are never written but feed the (zero-weighted)
            # reciprocal input; keep them finite
            nc.vector.memset(zb[pair][0:64, :, :], 1.0)
            step = 0
            for j in range(NTT):
                nsb_j = 4 * (j + 1) if causal else NSB
                outp = [pso.tile([D + 1, 512], F32, tag="pso",
                                 name=f"outp{i}") for i in range(2)]

                def emit_pv(i, lo, last):
                    for u in range(2):
                        nc.tensor.matmul(
                            outp[u][:, lo:512],
                            v_t[hg][:, i, pr * 2 + u, :],
                            pend[i][:, u, lo:512],
                            start=(i == 0), stop=last,
                            skip_group_check=True)
                    del pend[i]

                pend = {}
                prev = None
                for i in range(nsb_j):
                    r = i - 4 * j if causal else -1
                    lo = max(r, 0) * P
                    last = i == nsb_j - 1
                    scs = psA.tile([P, 2, 512], F32, tag="psA", name="scs")
                    pts = p_pool.tile([P, 2, 512], BF16, tag="pT", name="pts")
                    pend[i] = pts
                    for u in range(2):
                        dsl = slice(u * D, (u + 1) * D)
                        nc.tensor.matmul(
                            scs[:, u, lo:512],
                            kT2[pair][dsl, i * P:(i + 1) * P],
                            qT2[pair][dsl, j * 512 + lo:(j + 1) * 512],
                            start=True, stop=True)
                    nc.scalar.activation(
                        out=pts[:, :, lo:512],
                        in_=scs[:, :, lo:512],
                        func=mybir.ActivationFunctionType.Exp,
                        scale=SCALE)
                    if causal and r >= 0:
                        # zero the upper triangle of the diagonal block
                        # post-exp (GpSimd; keeps DVE/ScalarE free)
                        nc.gpsimd.affine_select(
                            out=pts[:, :, lo:lo + P],
                            in_=pts[:, :, lo:lo + P],
                            compare_op=mybir.AluOpType.is_ge,
                            fill=0.0, base=0,
                            pattern=[[0, 2], [1, P]], channel_multiplier=-1,
                        )
                    if prev is not None:
                        emit_pv(*prev)
                    prev = (i, lo, last)
                    step += 1
                    if fq and fill_every and step % fill_every == 0:
                        fq.popleft()()
                if prev is not None:
                    emit_pv(*prev)
                for u in range(2):
                    nc.vector.tensor_copy(
                        out=outcat[pair][u * D:(u + 1) * D,
                                         j * 512:(j + 1) * 512],
                        in_=outp[u][0:D, :])
                    nc.vector.tensor_copy(
                        out=zb[pair][32 * u:32 * u + 1, j, :],
                        in_=outp[u][D:D + 1, :])
                if on_j_done is not None:
                    on_j_done(j)

        # ================= schedule =================
        from functools import partial

        # P1: head-group 0 q/k projections, dense (ScalarE idle -> ACT
        # copies); v(hg0) s-window groups 0,1 emitted here so pair 0's
        # first PVs never wait.
        alloc_proj(0)
        for th in range(2):
            for pr in range(2):
                for which in range(2):
                    for tt in range(2):
                        qk_group(0, th, pr, which, tt, on_act=True)
        v_group(0, 0)
        v_group(0, 1)

        # P2: attention pairs 0,1; filler = rest of v(hg0), then all of
        # head-group 1's projections (v(hg0) first: s-paced consumption)
        alloc_proj(1)
        fq = deque()
        for g in range(2, 8):
            fq.append(partial(v_group, 0, g))
        for th in range(2):
            for pr in range(2):
                for which in range(2):
                    for tt in range(2):
                        fq.append(partial(qk_group, 1, th, pr, which, tt,
                                          False))
        attention_pair(0, fq, 3)
        attention_pair(1, fq, 2)
        while fq:
            fq.popleft()()
        emit_normalize(0)

        # P3a: pair 2; filler = v(hg1) groups (s-paced) + normalize(1)
        v_group(1, 0)
        v_group(1, 1)
        for g in range(2, 8):
            fq.append(partial(v_group, 1, g))
        fq.append(partial(emit_normalize, 1))
        attention_pair(2, fq, 4)
        while fq:
            fq.popleft()()

        # P3b: pair 3; per-j normalize + output-projection blocks feed the
        # PE while ScalarE works on the next j's exps
        rz[3] = rz_pool.tile([P, NTT, 512], BF16, tag="rz", name="rz3")
        fq.append(partial(emit_normalize, 2))

        def on_j_done(j):
            fq.append(partial(norm3_j, j))
            for m in range(4 * j, 4 * j + 4):
                for n in range(2):
                    fq.append(partial(yproj_group, m, n))
        attention_pair(3, fq, 1, on_j_done)
        while fq:
            fq.popleft()()


_NC_CACHE = {}
LAST_RESULTS = None


def kernel(x, Wq, Wk, Wv, Wp, bp, is_masked, **_unused):
    global LAST_RESULTS
    x = np.asarray(x, np.float32)
    Wq = np.asarray(Wq, np.float32)
    Wk = np.asarray(Wk, np.float32)
    Wv = np.asarray(Wv, np.float32)
    Wp = np.asarray(Wp, np.float32)
    bp = np.asarray(bp, np.float32)
    causal = bool(np.asarray(is_masked).item())

    if causal not in _NC_CACHE:
        _NC_CACHE[causal] = _build(causal)
    nc = _NC_CACHE[causal]

    # host-side layout prep (bf16)
    wq_r = np.ascontiguousarray(
        Wq.transpose(1, 0, 2).reshape(C, H * D)).astype(NPBF16)
    wk_r = np.ascontiguousarray(
        Wk.transpose(1, 0, 2).reshape(C, H * D)).astype(NPBF16)
    wv_r = np.ascontiguousarray(
        Wv.transpose(1, 0, 2).reshape(C, H * D)).astype(NPBF16)
    wpt = np.ascontiguousarray(Wp.T).astype(NPBF16)
    zeros = np.zeros_like(bp)

    sel = np.zeros((64, P), np.float32)
    sel[0, 0:64] = 1.0
    sel[32, 64:128] = 1.0
    sel = sel.astype(NPBF16)

    xTs = [np.ascontiguousarray(x[b].T).astype(NPBF16) for b in range(B)]
    in_maps = []
    for core in range(8):
        b, hh = core // 2, core % 2
        csl = slice(hh * HL * D, (hh + 1) * HL * D)
        in_maps.append({
            "xT": xTs[b],
            "wq": np.ascontiguousarray(wq_r[:, csl]),
            "wk": np.ascontiguousarray(wk_r[:, csl]),
            "wv": np.ascontiguousarray(wv_r[:, csl]),
            "wpt": np.ascontiguousarray(wpt[csl, :]),
            "bp": bp if hh == 0 else zeros,
            "sel": sel,
        })

    trace = bool(int(os.environ.get("KERNEL_TRACE", "0")))
    res = run_bass_kernel_spmd(
        nc, in_maps, core_ids=list(range(8)), trace=trace)
    LAST_RESULTS = res

    y = np.empty((B, T, C), np.float32)
    for b in range(B):
        y[b] = res.results[2 * b]["y"] + res.results[2 * b + 1]["y"]
    return y


# revision 35
# speedup vs baseline: 1.6184x; 1.0355x over previous
"""Multi-head attention (B=4, T=2048, C=1024, H=16, D=64) on 8 TRN2 cores.

Sharding: core i handles batch b=i//2 and the 8 heads of half hh=i%2.
Each core computes its heads' contribution through the row-sharded output
projection -> partial y [T, C]; host sums the two partials per batch.

v2: bf16 compute (FWL weight loads, no fp32 power throttle), software
pipelining to keep the PE HAM-warm:
  P1: dense q/k/v projections for head-group 0 (pairs 0,1)
  P2: attention pairs 0,1 with head-group 1's projection matmuls
      interleaved as PE filler between score steps (exp on ScalarE is
      the attention bottleneck; filler keeps the PE from micro-idling)
  P3: attention pairs 2,3 (normalize of earlier pairs interleaved)
  tail: last normalize + output projection
Causal mask applied post-exp on GpSimd (affine_select fill=0), score
matmuls N-trimmed on diagonal blocks, softmax normalizer via
reciprocal_approx_fast + K=2 selector broadcast matmul.

Per-core layouts (host pre-arranged, bf16):
  xT  [C, T]    = x[b].T
  wq/wk/wv [C, 512]  columns = (local head)*64 + d
  wpt [512, C]  rows  = (local head)*64 + d   (= Wp.T row-slice)
  bp  [C] f32   bias on even cores, zeros on odd (summed partials)
"""

import os
import sys

import numpy as np

for _p in ("/opt/trn_rl_repo", "/root/.axon_site/_ro/trn_rl_repo"):
    if os.path.isdir(_p) and _p not in sys.path:
        sys.path.append(_p)

import ml_dtypes

import concourse.bass as bass
import concourse.bacc as bacc
import concourse.mybir as mybir
import concourse.tile as tile
from concourse.bass_utils import run_bass_kernel_spmd

B, T, C, H, D = 4, 2048, 1024, 16, 64
HL = H // 2          # heads per core
P = 128
NCH = C // P         # 8 c-chunks
NTT = T // 512       # 4 t-tiles of 512
NSB = T // P         # 16 s-blocks of 128
SCALE = 1.0 / 32.0   # 1/sqrt(C)

F32 = mybir.dt.float32
F32R = mybir.dt.float32r
BF16 = mybir.dt.bfloat16
F8 = mybir.dt.float8e4
NPBF16 = ml_dtypes.bfloat16
NPF8 = ml_dtypes.float8_e4m3
W8SCALE = 64.0                     # q/k weights pre-scaled into fp8 range
SC_EXP8 = SCALE / (W8SCALE * W8SCALE)   # exp scale when q,k carry 64x


def _build(causal: bool) -> bass.Bass:
    nc = bacc.Bacc("TRN2", target_bir_lowering=False, debug=False, num_devices=8)

    xT = nc.dram_tensor("xT", [C, T], BF16, kind="ExternalInput").ap()
    xT8_d = nc.dram_tensor("xT8", [C, T], F8, kind="ExternalInput").ap()
    wq_d = nc.dram_tensor("wq", [C, HL * D], F8, kind="ExternalInput").ap()
    wk_d = nc.dram_tensor("wk", [C, HL * D], F8, kind="ExternalInput").ap()
    wv_d = nc.dram_tensor("wv", [C, HL * D], BF16, kind="ExternalInput").ap()
    wpt_d = nc.dram_tensor("wpt", [HL * D, C], BF16, kind="ExternalInput").ap()
    bp_d = nc.dram_tensor("bp", [C], F32, kind="ExternalInput").ap()
    sel_d = nc.dram_tensor("sel", [64, P], BF16, kind="ExternalInput").ap()
    y_d = nc.dram_tensor("y", [T, C], F32, kind="ExternalOutput").ap()

    with tile.TileContext(nc) as tc:
        _emit(nc, tc, causal, xT, xT8_d, wq_d, wk_d, wv_d, wpt_d, bp_d,
              sel_d, y_d)
    nc.compile()
    return nc


def _emit(nc, tc, causal, xT, xT8_d, wq_d, wk_d, wv_d, wpt_d, bp_d, sel_d,
          y_d):
    from collections import deque
    from contextlib import ExitStack

    ctx = ExitStack()
    with ctx:
        consts = ctx.enter_context(tc.tile_pool(name="consts", bufs=1))
        x_pool = ctx.enter_context(tc.tile_pool(name="xh", bufs=1))
        x8_pool = ctx.enter_context(tc.tile_pool(name="xh8", bufs=1))
        wq_pool = ctx.enter_context(tc.tile_pool(name="wq", bufs=2))
        wk_pool = ctx.enter_context(tc.tile_pool(name="wk", bufs=2))
        wv_pool = ctx.enter_context(tc.tile_pool(name="wv", bufs=2))
        q_pool = ctx.enter_context(tc.tile_pool(name="qT", bufs=4))
        k_pool = ctx.enter_context(tc.tile_pool(name="kT", bufs=4))
        v_pool = ctx.enter_context(tc.tile_pool(name="v", bufs=2))
        oc_pool = ctx.enter_context(tc.tile_pool(name="outcat", bufs=4))
        p_pool = ctx.enter_context(tc.tile_pool(name="pT", bufs=4))
        z_pool = ctx.enter_context(tc.tile_pool(name="zb", bufs=3))
        rz_pool = ctx.enter_context(tc.tile_pool(name="rz", bufs=2))
        rzs_pool = ctx.enter_context(tc.tile_pool(name="rzs", bufs=1))
        rzs3_pool = ctx.enter_context(tc.tile_pool(name="rzs3", bufs=2))
        yst_pool = ctx.enter_context(tc.tile_pool(name="yst", bufs=3))
        wpt_pool = ctx.enter_context(tc.tile_pool(name="wpt", bufs=4))
        bpb_pool = ctx.enter_context(tc.tile_pool(name="bpb", bufs=1))
        psA = ctx.enter_context(tc.tile_pool(name="psA", bufs=2, space="PSUM"))
        pso = ctx.enter_context(tc.tile_pool(name="pso", bufs=2, space="PSUM"))
        psP = ctx.enter_context(tc.tile_pool(name="psP", bufs=2, space="PSUM"))

        # Normalizer broadcast selector (host constant): row 0 -> out rows
        # 0-63 (u=0), row 32 -> out rows 64-127 (u=1); all other rows zero
        # so garbage in the unused rz partitions is multiplied by 0.
        sel = consts.tile([64, P], BF16)

        # ---- input DMAs; two HWDGE queues in parallel: sync carries the
        # fp8 q/k path (needed first), scalar carries the bf16 x / wv /
        # wpt / bpb path. Ordered so the first projection group and the
        # first PVs are ready ASAP. ----
        xh = x_pool.tile([P, NCH, T], BF16, tag="xh")
        xh8 = x8_pool.tile([P, NCH, T], F8, tag="xh8")
        wts = {}
        for hg in range(2):
            wq_t = wq_pool.tile([P, NCH, 4 * D], F8, tag="wq",
                                name=f"wq{hg}")
            wk_t = wk_pool.tile([P, NCH, 4 * D], F8, tag="wk",
                                name=f"wk{hg}")
            wv_t = wv_pool.tile([P, NCH, 4 * D], BF16, tag="wv",
                                name=f"wv{hg}")
            wts[hg] = (wq_t, wk_t, wv_t)

        def dma_w(hg, idx, eng):
            hsl = slice(hg * 4 * D, (hg + 1) * 4 * D)
            w_d = (wq_d, wk_d, wv_d)[idx]
            eng.dma_start(
                out=wts[hg][idx],
                in_=w_d[:, hsl].rearrange("(n p) d -> p n d", p=P))

        def dma_x(w, dst, src, eng):
            for c in range(NCH):
                eng.dma_start(
                    out=dst[:, c, w * 512:(w + 1) * 512],
                    in_=src[c * P:(c + 1) * P, w * 512:(w + 1) * 512])

        # sync queue: fp8 q/k inputs
        dma_w(0, 0, nc.sync)
        dma_w(0, 1, nc.sync)
        dma_x(0, xh8, xT8_d, nc.sync)
        for w in range(1, 4):
            dma_x(w, xh8, xT8_d, nc.sync)
        dma_w(1, 0, nc.sync)
        dma_w(1, 1, nc.sync)
        nc.sync.dma_start(out=sel, in_=sel_d)
        # scalar queue: bf16 v-path inputs + output-projection weights
        dma_x(0, xh, xT, nc.scalar)
        dma_w(0, 2, nc.scalar)
        for w in range(1, 4):
            dma_x(w, xh, xT, nc.scalar)
        dma_w(1, 2, nc.scalar)
        wpt_t = [wpt_pool.tile([P, C], BF16, tag="wpt", name=f"wpt{i}")
                 for i in range(4)]
        for q in range(4):
            nc.scalar.dma_start(out=wpt_t[q], in_=wpt_d[q * P:(q + 1) * P, :])
        bpb = bpb_pool.tile([P, C], F32)
        nc.scalar.dma_start(
            out=bpb,
            in_=bass.AP(tensor=bp_d.tensor, offset=0, ap=[[0, P], [1, C]]),
        )

        qT2 = [None] * 4   # per pair [128 (2 heads x 64d), T] bf16
        kT2 = [None] * 4
        v_t = [None] * 2   # per head-group [128 s, NSB, 4, D+1] bf16
        outcat = [oc_pool.tile([P, T], BF16, tag="outcat", name=f"outcat{i}")
                  for i in range(4)]
        zb = [None] * 4    # per pair [128, NTT, 512] f32; rows 0(u0)/32(u1)
        rz = [None] * 4

        # ---------------- projection pieces (filler-granular) ----------
        def alloc_proj(hg):
            for pr in range(2):
                pair = hg * 2 + pr
                qT2[pair] = q_pool.tile([P, T], BF16, tag="qT",
                                        name=f"qT{pair}")
                kT2[pair] = k_pool.tile([P, T], BF16, tag="kT",
                                        name=f"kT{pair}")
            v_t[hg] = v_pool.tile([P, NSB, 4, D + 1], BF16, tag="v",
                                  name=f"v{hg}")
            nc.vector.memset(v_t[hg][:, :, :, D:], 1.0)

        def qk_group(hg, th, pr, which, tt, on_act):
            # fp8 DoubleRow: each matmul contracts a pair of c-chunks
            # ([128, 2, .] APs); q/k tiles carry a 64x weight scale that
            # the exp scale divides back out.
            w_t = wts[hg][which]
            dst = (qT2 if which == 0 else kT2)[hg * 2 + pr]
            t0 = th * 1024 + tt * 512
            ps = psP.tile([P, 512], F32, tag="psP", name="qk")
            for g in range(NCH // 2):
                nc.tensor.matmul(
                    ps, w_t[:, 2 * g:2 * g + 2, pr * P:(pr + 1) * P],
                    xh8[:, 2 * g:2 * g + 2, t0:t0 + 512],
                    start=g == 0, stop=g == NCH // 2 - 1,
                    perf_mode=mybir.MatmulPerfMode.DoubleRow)
            if on_act:
                nc.scalar.copy(out=dst[:, t0:t0 + 512], in_=ps)
            else:
                nc.vector.tensor_copy(out=dst[:, t0:t0 + 512], in_=ps)

        def v_group(hg, g):   # g in 0..7 covers s [g*256, (g+1)*256)
            th, sbp = g // 4, g % 4
            wv_t = wts[hg][2]
            vps = psP.tile([P, 2, 256], F32, tag="psP", name="vps")
            for c in range(NCH):
                for u in range(2):
                    s0 = th * 1024 + (sbp * 2 + u) * P
                    nc.tensor.matmul(
                        vps[:, u, :],
                        xh[:, c, s0:s0 + P],
                        wts[hg][2][:, c, :],
                        start=(c == 0 and u == 0), stop=c == NCH - 1)
            sb0 = th * 8 + sbp * 2
            nc.vector.tensor_copy(
                out=v_t[hg][:, sb0:sb0 + 2, :, 0:D],
                in_=vps.rearrange("p u (h d) -> p u h d", h=4))

        # ---------------- normalize ----------------
        def emit_normalize(pair):
            rz[pair] = rz_pool.tile([P, NTT, 512], BF16, tag="rz",
                                    name=f"rz{pair}")
            scratch = rzs_pool.tile([64, NTT, 512], F32, tag="rzs")
            nc.vector.reciprocal_approx_fast(
                out=scratch, in_=zb[pair][0:64, :, :])
            nc.vector.tensor_copy(out=rz[pair][0:64, :, :], in_=scratch)
            for j in range(NTT):
                bps = psP.tile([P, 512], F32, tag="psP", name="bps")
                nc.tensor.matmul(bps, sel, rz[pair][0:64, j, :],
                                 start=True, stop=True)
                osl = outcat[pair][:, j * 512:(j + 1) * 512]
                nc.vector.tensor_mul(osl, osl, bps)

        def norm3_j(j):
            scratch = rzs3_pool.tile([64, 512], F32, tag="rzs3")
            nc.vector.reciprocal_approx_fast(
                out=scratch, in_=zb[3][0:64, j, :])
            nc.vector.tensor_copy(out=rz[3][0:64, j, :], in_=scratch)
            bps = psP.tile([P, 512], F32, tag="psP", name="bps")
            nc.tensor.matmul(bps, sel, rz[3][0:64, j, :],
                             start=True, stop=True)
            osl = outcat[3][:, j * 512:(j + 1) * 512]
            nc.vector.tensor_mul(osl, osl, bps)

        ydma_flip = [0]

        def yproj_group(m, n):
            yps = psP.tile([P, 512], F32, tag="psP", name="yps")
            for q in range(4):
                nc.tensor.matmul(
                    yps,
                    outcat[q][:, m * P:(m + 1) * P],
                    wpt_t[q][:, n * 512:(n + 1) * 512],
                    start=(q == 0), stop=(q == 3))
            yt = yst_pool.tile([P, 512], F32, tag="yst", name="yt")
            nc.vector.tensor_add(yt, yps, bpb[:, n * 512:(n + 1) * 512])
            eng = nc.sync if ydma_flip[0] % 2 == 0 else nc.scalar
            ydma_flip[0] += 1
            eng.dma_start(
                out=y_d[m * P:(m + 1) * P, n * 512:(n + 1) * 512],
                in_=yt)

        # ---------------- attention ----------------
        def alloc_zb(pair):
            zb[pair] = z_pool.tile([P, NTT, 512], F32, tag="zb",
                                   name=f"zb{pair}")
            # rows 1-31/33-63 are never written but feed the (zero-weighted)
            # reciprocal input; keep them finite
            nc.vector.memset(zb[pair][0:64, :, :], 1.0)

        def attention_pair(pair, fq, fill_every, on_j_done=None):
            hg, pr = pair // 2, pair % 2
            step = 0
            for j in range(NTT):
                nsb_j = 4 * (j + 1) if causal else NSB
                outp = [pso.tile([D + 1, 512], F32, tag="pso",
                                 name=f"outp{i}") for i in range(2)]

                def emit_pv(i, lo, last):
                    for u in range(2):
                        nc.tensor.matmul(
                            outp[u][:, lo:512],
                            v_t[hg][:, i, pr * 2 + u, :],
                            pend[i][:, u, lo:512],
                            start=(i == 0), stop=last,
                            skip_group_check=True)
                    del pend[i]

                pend = {}
                prev = None
                for i in range(nsb_j):
                    r = i - 4 * j if causal else -1
                    lo = max(r, 0) * P
                    last = i == nsb_j - 1
                    scs = psA.tile([P, 2, 512], F32, tag="psA", name="scs")
                    pts = p_pool.tile([P, 2, 512], BF16, tag="pT", name="pts")
                    pend[i] = pts
                    for u in range(2):
                        dsl = slice(u * D, (u + 1) * D)
                        nc.tensor.matmul(
                            scs[:, u, lo:512],
                            kT2[pair][dsl, i * P:(i + 1) * P],
                            qT2[pair][dsl, j * 512 + lo:(j + 1) * 512],
                            start=True, stop=True)
                    nc.scalar.activation(
                        out=pts[:, :, lo:512],
                        in_=scs[:, :, lo:512],
                        func=mybir.ActivationFunctionType.Exp,
                        scale=SC_EXP8)
                    if causal and r >= 0:
                        # zero the upper triangle of the diagonal block
                        # post-exp (GpSimd; keeps DVE/ScalarE free)
                        nc.gpsimd.affine_select(
                            out=pts[:, :, lo:lo + P],
                            in_=pts[:, :, lo:lo + P],
                            compare_op=mybir.AluOpType.is_ge,
                            fill=0.0, base=0,
                            pattern=[[0, 2], [1, P]], channel_multiplier=-1,
                        )
                    if prev is not None:
                        emit_pv(*prev)
                    prev = (i, lo, last)
                    step += 1
                    if fq and fill_every and step % fill_every == 0:
                        fq.popleft()()
                if prev is not None:
                    emit_pv(*prev)
                for u in range(2):
                    nc.vector.tensor_copy(
                        out=outcat[pair][u * D:(u + 1) * D,
                                         j * 512:(j + 1) * 512],
                        in_=outp[u][0:D, :])
                    nc.vector.tensor_copy(
                        out=zb[pair][32 * u:32 * u + 1, j, :],
                        in_=outp[u][D:D + 1, :])
                if on_j_done is not None:
                    on_j_done(j)

        # ================= schedule =================
        from functools import partial

        # P1: head-group 0 q/k projections, dense, ordered by x window so
        # the PE starts as soon as window 0 lands and never gaps (HAM
        # warm-up); copies alternate ACT/DVE. v(hg0) windows 0,1 emitted
        # here so pair 0's first PVs never wait.
        alloc_proj(0)
        flip = 0
        for th in range(2):
            for tt in range(2):
                for pr in range(2):
                    for which in range(2):
                        qk_group(0, th, pr, which, tt, on_act=flip % 2 == 0)
                        flip += 1
        v_group(0, 0)
        v_group(0, 1)

        # P2: attention pairs 0,1; filler = rest of v(hg0), then all of
        # head-group 1's projections (v(hg0) first: s-paced consumption)
        alloc_proj(1)
        alloc_zb(0)
        alloc_zb(1)
        fq = deque()
        for g in range(2, 8):
            fq.append(partial(v_group, 0, g))
        for th in range(2):
            for tt in range(2):
                for pr in range(2):
                    for which in range(2):
                        fq.append(partial(qk_group, 1, th, pr, which, tt,
                                          False))
        attention_pair(0, fq, 3)
        attention_pair(1, fq, 2)
        while fq:
            fq.popleft()()
        emit_normalize(0)

        # P3a: pair 2; filler = v(hg1) groups (s-paced) + normalize(1)
        alloc_zb(2)
        alloc_zb(3)
        v_group(1, 0)
        v_group(1, 1)
        for g in range(2, 8):
            fq.append(partial(v_group, 1, g))
        fq.append(partial(emit_normalize, 1))
        attention_pair(2, fq, 4)
        while fq:
            fq.popleft()()

        # P3b: pair 3; per-j normalize + output-projection blocks feed the
        # PE while ScalarE works on the next j's exps
        rz[3] = rz_pool.tile([P, NTT, 512], BF16, tag="rz", name="rz3")
        fq.append(partial(emit_normalize, 2))

        def on_j_done(j):
            fq.append(partial(norm3_j, j))
            for m in range(4 * j, 4 * j + 4):
                for n in range(2):
                    fq.append(partial(yproj_group, m, n))
        attention_pair(3, fq, 2, on_j_done)
        while fq:
            fq.popleft()()


_NC_CACHE = {}
LAST_RESULTS = None


def kernel(x, Wq, Wk, Wv, Wp, bp, is_masked, **_unused):
    global LAST_RESULTS
    x = np.asarray(x, np.float32)
    Wq = np.asarray(Wq, np.float32)
    Wk = np.asarray(Wk, np.float32)
    Wv = np.asarray(Wv, np.float32)
    Wp = np.asarray(Wp, np.float32)
    bp = np.asarray(bp, np.float32)
    causal = bool(np.asarray(is_masked).item())

    if causal not in _NC_CACHE:
        _NC_CACHE[causal] = _build(causal)
    nc = _NC_CACHE[causal]

    # host-side layout prep (bf16 for v/output path, fp8 for q/k path)
    wq_r = np.ascontiguousarray(
        Wq.transpose(1, 0, 2).reshape(C, H * D) * W8SCALE).astype(NPF8)
    wk_r = np.ascontiguousarray(
        Wk.transpose(1, 0, 2).reshape(C, H * D) * W8SCALE).astype(NPF8)
    wv_r = np.ascontiguousarray(
        Wv.transpose(1, 0, 2).reshape(C, H * D)).astype(NPBF16)
    wpt = np.ascontiguousarray(Wp.T).astype(NPBF16)
    zeros = np.zeros_like(bp)

    sel = np.zeros((64, P), np.float32)
    sel[0, 0:64] = 1.0
    sel[32, 64:128] = 1.0
    sel = sel.astype(NPBF16)

    xTs = [np.ascontiguousarray(x[b].T).astype(NPBF16) for b in range(B)]
    xTs8 = [np.ascontiguousarray(x[b].T).astype(NPF8) for b in range(B)]
    in_maps = []
    for core in range(8):
        b, hh = core // 2, core % 2
        csl = slice(hh * HL * D, (hh + 1) * HL * D)
        in_maps.append({
            "xT": xTs[b],
            "xT8": xTs8[b],
            "wq": np.ascontiguousarray(wq_r[:, csl]),
            "wk": np.ascontiguousarray(wk_r[:, csl]),
            "wv": np.ascontiguousarray(wv_r[:, csl]),
            "wpt": np.ascontiguousarray(wpt[csl, :]),
            "bp": bp if hh == 0 else zeros,
            "sel": sel,
        })

    trace = bool(int(os.environ.get("KERNEL_TRACE", "0")))
    res = run_bass_kernel_spmd(
        nc, in_maps, core_ids=list(range(8)), trace=trace)
    LAST_RESULTS = res

    y = np.empty((B, T, C), np.float32)
    for b in range(B):
        y[b] = res.results[2 * b]["y"] + res.results[2 * b + 1]["y"]
    return y


# revision 36
# speedup vs baseline: 1.7253x; 1.0661x over previous
"""Multi-head attention (B=4, T=2048, C=1024, H=16, D=64) on 8 TRN2 cores.

Sharding: core i handles batch b=i//2 and the 8 heads of half hh=i%2.
Each core computes its heads' contribution through the row-sharded output
projection -> partial y [T, C]; host sums the two partials per batch.

v2: bf16 compute (FWL weight loads, no fp32 power throttle), software
pipelining to keep the PE HAM-warm:
  P1: dense q/k/v projections for head-group 0 (pairs 0,1)
  P2: attention pairs 0,1 with head-group 1's projection matmuls
      interleaved as PE filler between score steps (exp on ScalarE is
      the attention bottleneck; filler keeps the PE from micro-idling)
  P3: attention pairs 2,3 (normalize of earlier pairs interleaved)
  tail: last normalize + output projection
Causal mask applied post-exp on GpSimd (affine_select fill=0), score
matmuls N-trimmed on diagonal blocks, softmax normalizer via
reciprocal_approx_fast + K=2 selector broadcast matmul.

Per-core layouts (host pre-arranged, bf16):
  xT  [C, T]    = x[b].T
  wq/wk/wv [C, 512]  columns = (local head)*64 + d
  wpt [512, C]  rows  = (local head)*64 + d   (= Wp.T row-slice)
  bp  [C] f32   bias on even cores, zeros on odd (summed partials)
"""

import os
import sys

import numpy as np

for _p in ("/opt/trn_rl_repo", "/root/.axon_site/_ro/trn_rl_repo"):
    if os.path.isdir(_p) and _p not in sys.path:
        sys.path.append(_p)

import ml_dtypes

import concourse.bass as bass
import concourse.bacc as bacc
import concourse.mybir as mybir
import concourse.tile as tile
from concourse.bass_utils import run_bass_kernel_spmd

B, T, C, H, D = 4, 2048, 1024, 16, 64
HL = H // 2          # heads per core
P = 128
NCH = C // P         # 8 c-chunks
NTT = T // 512       # 4 t-tiles of 512
NSB = T // P         # 16 s-blocks of 128
SCALE = 1.0 / 32.0   # 1/sqrt(C)

F32 = mybir.dt.float32
F32R = mybir.dt.float32r
BF16 = mybir.dt.bfloat16
F8 = mybir.dt.float8e4
NPBF16 = ml_dtypes.bfloat16
NPF8 = ml_dtypes.float8_e4m3
W8SCALE = 64.0                     # q/k weights pre-scaled into fp8 range
SC_EXP8 = SCALE / (W8SCALE * W8SCALE)   # exp scale when q,k carry 64x


def _build(causal: bool) -> bass.Bass:
    nc = bacc.Bacc("TRN2", target_bir_lowering=False, debug=False, num_devices=8)

    xT = nc.dram_tensor("xT", [C, T], BF16, kind="ExternalInput").ap()
    xT8_d = nc.dram_tensor("xT8", [C, T], F8, kind="ExternalInput").ap()
    wq_d = nc.dram_tensor("wq", [C, HL * D], F8, kind="ExternalInput").ap()
    wk_d = nc.dram_tensor("wk", [C, HL * D], F8, kind="ExternalInput").ap()
    wv_d = nc.dram_tensor("wv", [C, HL * D], BF16, kind="ExternalInput").ap()
    wpt_d = nc.dram_tensor("wpt", [HL * D, C], BF16, kind="ExternalInput").ap()
    bp_d = nc.dram_tensor("bp", [C], F32, kind="ExternalInput").ap()
    sel_d = nc.dram_tensor("sel", [64, P], BF16, kind="ExternalInput").ap()
    y_d = nc.dram_tensor("y", [T, C], F32, kind="ExternalOutput").ap()

    with tile.TileContext(nc) as tc:
        _emit(nc, tc, causal, xT, xT8_d, wq_d, wk_d, wv_d, wpt_d, bp_d,
              sel_d, y_d)
    nc.compile()
    return nc


def _emit(nc, tc, causal, xT, xT8_d, wq_d, wk_d, wv_d, wpt_d, bp_d, sel_d,
          y_d):
    from collections import deque
    from contextlib import ExitStack

    ctx = ExitStack()
    with ctx:
        consts = ctx.enter_context(tc.tile_pool(name="consts", bufs=1))
        x_pool = ctx.enter_context(tc.tile_pool(name="xh", bufs=1))
        x8_pool = ctx.enter_context(tc.tile_pool(name="xh8", bufs=1))
        wq_pool = ctx.enter_context(tc.tile_pool(name="wq", bufs=2))
        wk_pool = ctx.enter_context(tc.tile_pool(name="wk", bufs=2))
        wv_pool = ctx.enter_context(tc.tile_pool(name="wv", bufs=2))
        q_pool = ctx.enter_context(tc.tile_pool(name="qT", bufs=4))
        k_pool = ctx.enter_context(tc.tile_pool(name="kT", bufs=4))
        v_pool = ctx.enter_context(tc.tile_pool(name="v", bufs=2))
        oc_pool = ctx.enter_context(tc.tile_pool(name="outcat", bufs=4))
        p_pool = ctx.enter_context(tc.tile_pool(name="pT", bufs=4))
        z_pool = ctx.enter_context(tc.tile_pool(name="zb", bufs=3))
        rz_pool = ctx.enter_context(tc.tile_pool(name="rz", bufs=2))
        rzs_pool = ctx.enter_context(tc.tile_pool(name="rzs", bufs=1))
        rzs3_pool = ctx.enter_context(tc.tile_pool(name="rzs3", bufs=2))
        yst_pool = ctx.enter_context(tc.tile_pool(name="yst", bufs=3))
        wpt_pool = ctx.enter_context(tc.tile_pool(name="wpt", bufs=4))
        bpb_pool = ctx.enter_context(tc.tile_pool(name="bpb", bufs=1))
        psA = ctx.enter_context(tc.tile_pool(name="psA", bufs=2, space="PSUM"))
        pso = ctx.enter_context(tc.tile_pool(name="pso", bufs=2, space="PSUM"))
        psP = ctx.enter_context(tc.tile_pool(name="psP", bufs=2, space="PSUM"))

        # Normalizer broadcast selector (host constant): row 0 -> out rows
        # 0-63 (u=0), row 32 -> out rows 64-127 (u=1); all other rows zero
        # so garbage in the unused rz partitions is multiplied by 0.
        sel = consts.tile([64, P], BF16)

        # ---- input DMAs; two HWDGE queues in parallel: sync carries the
        # fp8 q/k path (needed first), scalar carries the bf16 x / wv /
        # wpt / bpb path. Ordered so the first projection group and the
        # first PVs are ready ASAP. ----
        xh = x_pool.tile([P, NCH, T], BF16, tag="xh")
        xh8 = x8_pool.tile([P, NCH, T], F8, tag="xh8")
        wts = {}
        for hg in range(2):
            wq_t = wq_pool.tile([P, NCH, 4 * D], F8, tag="wq",
                                name=f"wq{hg}")
            wk_t = wk_pool.tile([P, NCH, 4 * D], F8, tag="wk",
                                name=f"wk{hg}")
            wv_t = wv_pool.tile([P, NCH, 4 * D], BF16, tag="wv",
                                name=f"wv{hg}")
            wts[hg] = (wq_t, wk_t, wv_t)

        def dma_w(hg, idx, eng):
            hsl = slice(hg * 4 * D, (hg + 1) * 4 * D)
            w_d = (wq_d, wk_d, wv_d)[idx]
            eng.dma_start(
                out=wts[hg][idx],
                in_=w_d[:, hsl].rearrange("(n p) d -> p n d", p=P))

        def dma_x(w, dst, src, eng):
            for c in range(NCH):
                eng.dma_start(
                    out=dst[:, c, w * 512:(w + 1) * 512],
                    in_=src[c * P:(c + 1) * P, w * 512:(w + 1) * 512])

        # sync queue (SP engine only -- ACT/GpSimd streams must stay free
        # for compute): fp8 q/k path first, bf16 v path interleaved in
        # consumption order
        dma_w(0, 0, nc.sync)
        dma_w(0, 1, nc.sync)
        dma_x(0, xh8, xT8_d, nc.sync)
        dma_x(1, xh8, xT8_d, nc.sync)
        dma_x(0, xh, xT, nc.sync)
        dma_w(0, 2, nc.sync)
        dma_x(2, xh8, xT8_d, nc.sync)
        dma_x(3, xh8, xT8_d, nc.sync)
        for w in range(1, 4):
            dma_x(w, xh, xT, nc.sync)
        dma_w(1, 0, nc.sync)
        dma_w(1, 1, nc.sync)
        dma_w(1, 2, nc.sync)
        nc.sync.dma_start(out=sel, in_=sel_d)
        # gpsimd software DGE: late-needed output-projection constants
        wpt_t = [wpt_pool.tile([P, C], BF16, tag="wpt", name=f"wpt{i}")
                 for i in range(4)]
        for q in range(4):
            nc.gpsimd.dma_start(out=wpt_t[q], in_=wpt_d[q * P:(q + 1) * P, :])
        bpb = bpb_pool.tile([P, C], F32)
        nc.gpsimd.dma_start(
            out=bpb,
            in_=bass.AP(tensor=bp_d.tensor, offset=0, ap=[[0, P], [1, C]]),
        )

        qT2 = [None] * 4   # per pair [128 (2 heads x 64d), T] bf16
        kT2 = [None] * 4
        v_t = [None] * 2   # per head-group [128 s, NSB, 4, D+1] bf16
        outcat = [oc_pool.tile([P, T], BF16, tag="outcat", name=f"outcat{i}")
                  for i in range(4)]
        zb = [None] * 4    # per pair [128, NTT, 512] f32; rows 0(u0)/32(u1)
        rz = [None] * 4

        # ---------------- projection pieces (filler-granular) ----------
        def alloc_proj(hg):
            for pr in range(2):
                pair = hg * 2 + pr
                qT2[pair] = q_pool.tile([P, T], BF16, tag="qT",
                                        name=f"qT{pair}")
                kT2[pair] = k_pool.tile([P, T], BF16, tag="kT",
                                        name=f"kT{pair}")
            v_t[hg] = v_pool.tile([P, NSB, 4, D + 1], BF16, tag="v",
                                  name=f"v{hg}")
            nc.vector.memset(v_t[hg][:, :, :, D:], 1.0)

        def qk_group(hg, th, pr, which, tt, on_act):
            # fp8 DoubleRow: each matmul contracts a pair of c-chunks
            # ([128, 2, .] APs); q/k tiles carry a 64x weight scale that
            # the exp scale divides back out.
            w_t = wts[hg][which]
            dst = (qT2 if which == 0 else kT2)[hg * 2 + pr]
            t0 = th * 1024 + tt * 512
            ps = psP.tile([P, 512], F32, tag="psP", name="qk")
            for g in range(NCH // 2):
                nc.tensor.matmul(
                    ps, w_t[:, 2 * g:2 * g + 2, pr * P:(pr + 1) * P],
                    xh8[:, 2 * g:2 * g + 2, t0:t0 + 512],
                    start=g == 0, stop=g == NCH // 2 - 1,
                    perf_mode=mybir.MatmulPerfMode.DoubleRow)
            if on_act:
                nc.scalar.copy(out=dst[:, t0:t0 + 512], in_=ps)
            else:
                nc.vector.tensor_copy(out=dst[:, t0:t0 + 512], in_=ps)

        def v_group(hg, g):   # g in 0..7 covers s [g*256, (g+1)*256)
            th, sbp = g // 4, g % 4
            wv_t = wts[hg][2]
            vps = psP.tile([P, 2, 256], F32, tag="psP", name="vps")
            for c in range(NCH):
                for u in range(2):
                    s0 = th * 1024 + (sbp * 2 + u) * P
                    nc.tensor.matmul(
                        vps[:, u, :],
                        xh[:, c, s0:s0 + P],
                        wts[hg][2][:, c, :],
                        start=(c == 0 and u == 0), stop=c == NCH - 1)
            sb0 = th * 8 + sbp * 2
            nc.vector.tensor_copy(
                out=v_t[hg][:, sb0:sb0 + 2, :, 0:D],
                in_=vps.rearrange("p u (h d) -> p u h d", h=4))

        # ---------------- normalize ----------------
        def emit_normalize(pair):
            rz[pair] = rz_pool.tile([P, NTT, 512], BF16, tag="rz",
                                    name=f"rz{pair}")
            scratch = rzs_pool.tile([64, NTT, 512], F32, tag="rzs")
            nc.vector.reciprocal_approx_fast(
                out=scratch, in_=zb[pair][0:64, :, :])
            nc.vector.tensor_copy(out=rz[pair][0:64, :, :], in_=scratch)
            for j in range(NTT):
                bps = psP.tile([P, 512], F32, tag="psP", name="bps")
                nc.tensor.matmul(bps, sel, rz[pair][0:64, j, :],
                                 start=True, stop=True)
                osl = outcat[pair][:, j * 512:(j + 1) * 512]
                nc.vector.tensor_mul(osl, osl, bps)

        def norm3_j(j):
            scratch = rzs3_pool.tile([64, 512], F32, tag="rzs3")
            nc.vector.reciprocal_approx_fast(
                out=scratch, in_=zb[3][0:64, j, :])
            nc.vector.tensor_copy(out=rz[3][0:64, j, :], in_=scratch)
            bps = psP.tile([P, 512], F32, tag="psP", name="bps")
            nc.tensor.matmul(bps, sel, rz[3][0:64, j, :],
                             start=True, stop=True)
            osl = outcat[3][:, j * 512:(j + 1) * 512]
            nc.vector.tensor_mul(osl, osl, bps)

        ydma_flip = [0]

        def yproj_group(m, n):
            yps = psP.tile([P, 512], F32, tag="psP", name="yps")
            for q in range(4):
                nc.tensor.matmul(
                    yps,
                    outcat[q][:, m * P:(m + 1) * P],
                    wpt_t[q][:, n * 512:(n + 1) * 512],
                    start=(q == 0), stop=(q == 3))
            yt = yst_pool.tile([P, 512], F32, tag="yst", name="yt")
            nc.vector.tensor_add(yt, yps, bpb[:, n * 512:(n + 1) * 512])
            eng = nc.sync if ydma_flip[0] % 2 == 0 else nc.scalar
            ydma_flip[0] += 1
            eng.dma_start(
                out=y_d[m * P:(m + 1) * P, n * 512:(n + 1) * 512],
                in_=yt)

        # ---------------- attention ----------------
        def alloc_zb(pair):
            zb[pair] = z_pool.tile([P, NTT, 512], F32, tag="zb",
                                   name=f"zb{pair}")
            # rows 1-31/33-63 are never written but feed the (zero-weighted)
            # reciprocal input; keep them finite
            nc.vector.memset(zb[pair][0:64, :, :], 1.0)

        def attention_pair(pair, fq, fill_every, on_j_done=None):
            hg, pr = pair // 2, pair % 2
            step = 0
            for j in range(NTT):
                nsb_j = 4 * (j + 1) if causal else NSB
                outp = [pso.tile([D + 1, 512], F32, tag="pso",
                                 name=f"outp{i}") for i in range(2)]

                def emit_pv(i, lo, last):
                    for u in range(2):
                        nc.tensor.matmul(
                            outp[u][:, lo:512],
                            v_t[hg][:, i, pr * 2 + u, :],
                            pend[i][:, u, lo:512],
                            start=(i == 0), stop=last,
                            skip_group_check=True)
                    del pend[i]

                pend = {}
                prev = None
                for i in range(nsb_j):
                    r = i - 4 * j if causal else -1
                    lo = max(r, 0) * P
                    last = i == nsb_j - 1
                    scs = psA.tile([P, 2, 512], F32, tag="psA", name="scs")
                    pts = p_pool.tile([P, 2, 512], BF16, tag="pT", name="pts")
                    pend[i] = pts
                    for u in range(2):
                        dsl = slice(u * D, (u + 1) * D)
                        nc.tensor.matmul(
                            scs[:, u, lo:512],
                            kT2[pair][dsl, i * P:(i + 1) * P],
                            qT2[pair][dsl, j * 512 + lo:(j + 1) * 512],
                            start=True, stop=True)
                    nc.scalar.activation(
                        out=pts[:, :, lo:512],
                        in_=scs[:, :, lo:512],
                        func=mybir.ActivationFunctionType.Exp,
                        scale=SC_EXP8)
                    if causal and r >= 0:
                        # zero the upper triangle of the diagonal block
                        # post-exp (GpSimd; keeps DVE/ScalarE free)
                        nc.gpsimd.affine_select(
                            out=pts[:, :, lo:lo + P],
                            in_=pts[:, :, lo:lo + P],
                            compare_op=mybir.AluOpType.is_ge,
                            fill=0.0, base=0,
                            pattern=[[0, 2], [1, P]], channel_multiplier=-1,
                        )
                    if prev is not None:
                        emit_pv(*prev)
                    prev = (i, lo, last)
                    step += 1
                    if fq and fill_every and step % fill_every == 0:
                        fq.popleft()()
                if prev is not None:
                    emit_pv(*prev)
                for u in range(2):
                    nc.vector.tensor_copy(
                        out=outcat[pair][u * D:(u + 1) * D,
                                         j * 512:(j + 1) * 512],
                        in_=outp[u][0:D, :])
                    nc.vector.tensor_copy(
                        out=zb[pair][32 * u:32 * u + 1, j, :],
                        in_=outp[u][D:D + 1, :])
                if on_j_done is not None:
                    on_j_done(j)

        # ================= schedule =================
        from functools import partial

        # P1: head-group 0 q/k projections, dense, ordered by x window so
        # the PE starts as soon as window 0 lands and never gaps (HAM
        # warm-up); copies alternate ACT/DVE. v(hg0) windows 0,1 emitted
        # here so pair 0's first PVs never wait.
        alloc_proj(0)
        flip = 0
        for th in range(2):
            for tt in range(2):
                for pr in range(2):
                    for which in range(2):
                        qk_group(0, th, pr, which, tt, on_act=flip % 2 == 0)
                        flip += 1
        v_group(0, 0)
        v_group(0, 1)

        # P2: attention pairs 0,1; filler = rest of v(hg0), then all of
        # head-group 1's projections (v(hg0) first: s-paced consumption)
        alloc_proj(1)
        alloc_zb(0)
        alloc_zb(1)
        fq = deque()
        for g in range(2, 8):
            fq.append(partial(v_group, 0, g))
        for th in range(2):
            for tt in range(2):
                for pr in range(2):
                    for which in range(2):
                        fq.append(partial(qk_group, 1, th, pr, which, tt,
                                          False))
        attention_pair(0, fq, 3)
        attention_pair(1, fq, 2)
        while fq:
            fq.popleft()()
        emit_normalize(0)

        # P3a: pair 2; filler = v(hg1) groups (s-paced) + normalize(1)
        alloc_zb(2)
        alloc_zb(3)
        v_group(1, 0)
        v_group(1, 1)
        for g in range(2, 8):
            fq.append(partial(v_group, 1, g))
        fq.append(partial(emit_normalize, 1))
        attention_pair(2, fq, 4)
        while fq:
            fq.popleft()()

        # P3b: pair 3; per-j normalize + output-projection blocks feed the
        # PE while ScalarE works on the next j's exps
        rz[3] = rz_pool.tile([P, NTT, 512], BF16, tag="rz", name="rz3")
        fq.append(partial(emit_normalize, 2))

        def on_j_done(j):
            fq.append(partial(norm3_j, j))
            for m in range(4 * j, 4 * j + 4):
                for n in range(2):
                    fq.append(partial(yproj_group, m, n))
        attention_pair(3, fq, 2, on_j_done)
        while fq:
            fq.popleft()()


_NC_CACHE = {}
LAST_RESULTS = None


def kernel(x, Wq, Wk, Wv, Wp, bp, is_masked, **_unused):
    global LAST_RESULTS
    x = np.asarray(x, np.float32)
    Wq = np.asarray(Wq, np.float32)
    Wk = np.asarray(Wk, np.float32)
    Wv = np.asarray(Wv, np.float32)
    Wp = np.asarray(Wp, np.float32)
    bp = np.asarray(bp, np.float32)
    causal = bool(np.asarray(is_masked).item())

    if causal not in _NC_CACHE:
        _NC_CACHE[causal] = _build(causal)
    nc = _NC_CACHE[causal]

    # host-side layout prep (bf16 for v/output path, fp8 for q/k path)
    wq_r = np.ascontiguousarray(
        Wq.transpose(1, 0, 2).reshape(C, H * D) * W8SCALE).astype(NPF8)
    wk_r = np.ascontiguousarray(
        Wk.transpose(1, 0, 2).reshape(C, H * D) * W8SCALE).astype(NPF8)
    wv_r = np.ascontiguousarray(
        Wv.transpose(1, 0, 2).reshape(C, H * D)).astype(NPBF16)
    wpt = np.ascontiguousarray(Wp.T).astype(NPBF16)
    zeros = np.zeros_like(bp)

    sel = np.zeros((64, P), np.float32)
    sel[0, 0:64] = 1.0
    sel[32, 64:128] = 1.0
    sel = sel.astype(NPBF16)

    xTs = [np.ascontiguousarray(x[b].T).astype(NPBF16) for b in range(B)]
    xTs8 = [np.ascontiguousarray(x[b].T).astype(NPF8) for b in range(B)]
    in_maps = []
    for core in range(8):
        b, hh = core // 2, core % 2
        csl = slice(hh * HL * D, (hh + 1) * HL * D)
        in_maps.append({
            "xT": xTs[b],
            "xT8": xTs8[b],
            "wq": np.ascontiguousarray(wq_r[:, csl]),
            "wk": np.ascontiguousarray(wk_r[:, csl]),
            "wv": np.ascontiguousarray(wv_r[:, csl]),
            "wpt": np.ascontiguousarray(wpt[csl, :]),
            "bp": bp if hh == 0 else zeros,
            "sel": sel,
        })

    trace = bool(int(os.environ.get("KERNEL_TRACE", "0")))
    res = run_bass_kernel_spmd(
        nc, in_maps, core_ids=list(range(8)), trace=trace)
    LAST_RESULTS = res

    y = np.empty((B, T, C), np.float32)
    for b in range(B):
        y[b] = res.results[2 * b]["y"] + res.results[2 * b + 1]["y"]
    return y
